# revision 9
# baseline (speedup 1.0000x reference)
"""AttentivePoolingNetwork TRN2 kernel.

Data-parallel over batch across 8 NeuronCores (16 samples/core).
Per sample:
  Q = conv1d(emb[questions]) + b    [F=400, LQ=128]   (fp32r matmuls)
  A = conv1d(emb[answers]) + b      [F=400, LA=512]
  G = tanh(Q^T W A)                 [128, 512]
  wQ = exp(max_a G)  (softmax normalization cancels in the final cosine)
  wA = exp(max_q G)
  out = cos(Q wQ, A wA)
"""
import numpy as np
import concourse.bass as bass
import concourse.tile as tile
from concourse import bacc, mybir
from concourse.bass_utils import run_bass_kernel_spmd
from concourse.masks import make_identity

f32 = mybir.dt.float32
f32r = mybir.dt.float32r
i32 = mybir.dt.int32
AX = mybir.AxisListType.X
AF = mybir.ActivationFunctionType

B, LQ, LA = 128, 128, 512
VOCAB, EMB, FILT, K = 50000, 300, 400, 3
NCORES = 8
BC = B // NCORES        # 16 samples per core
EC = 100                # e-chunk size -> 3 chunks
FC = 100                # f-chunk size -> 4 chunks
NEC = EMB // EC         # 3
NFC = FILT // FC        # 4
GRP = 4                 # samples per Q-group (packs Q-conv/Z1 to N=512)
EPS = 1e-8
DEBUG = False
DBG_B = 1


def _build():
    nc = bacc.Bacc("TRN2", target_bir_lowering=False, debug=False, num_devices=NCORES)

    q_d = nc.declare_dram_parameter("questions", [BC, LQ], i32, isOutput=False)
    a_d = nc.declare_dram_parameter("answers", [BC, LA], i32, isOutput=False)
    emb_d = nc.declare_dram_parameter("emb", [VOCAB + 1, EMB], f32, isOutput=False)
    # host-prepped: cwt[k*3+c, e', f] = conv_w[f, 100c+e', k]
    cwt_d = nc.declare_dram_parameter("cwt", [K * NEC, EC, FILT], f32, isOutput=False)
    w_d = nc.declare_dram_parameter("W", [FILT, FILT], f32, isOutput=False)
    b_d = nc.declare_dram_parameter("conv_b", [FILT], f32, isOutput=False)
    ones_d = nc.declare_dram_parameter("ones", [128, 128], f32, isOutput=False)
    out_d = nc.declare_dram_parameter("out", [BC, 1], f32, isOutput=True)
    if DEBUG:
        xta_dbg = nc.declare_dram_parameter("xta_dbg", [128, NEC, LA + 2], f32, isOutput=True)
        asb_dbg = nc.declare_dram_parameter("asb_dbg", [128, NFC, LA], f32, isOutput=True)
        gs_dbg = nc.declare_dram_parameter("gs_dbg", [128, 512], f32, isOutput=True)
        wa_dbg = nc.declare_dram_parameter("wa_dbg", [128, 4], f32, isOutput=True)
        wq_dbg = nc.declare_dram_parameter("wq_dbg", [128, 1], f32, isOutput=True)
        at_dbg = nc.declare_dram_parameter("at_dbg", [128, 4, FILT], f32, isOutput=True)
        qt_dbg = nc.declare_dram_parameter("qt_dbg", [128, FILT], f32, isOutput=True)
        rq_dbg = nc.declare_dram_parameter("rq_dbg", [128, BC, NFC], f32, isOutput=True)
        ra_dbg = nc.declare_dram_parameter("ra_dbg", [128, BC, NFC], f32, isOutput=True)
        qg_dbg = nc.declare_dram_parameter("qg_dbg", [128, NFC, GRP, LQ], f32, isOutput=True)
        z1_dbg = nc.declare_dram_parameter("z1_dbg", [128, NFC, GRP, LQ], f32, isOutput=True)

    with tile.TileContext(nc) as tc:
        with (
            tc.tile_pool(name="const", bufs=1) as cst,
            tc.tile_pool(name="io", bufs=2) as io,
            tc.tile_pool(name="xt", bufs=2) as xt,
            tc.tile_pool(name="enc", bufs=2) as enc,
            tc.tile_pool(name="gp", bufs=2) as gp,
            tc.tile_pool(name="fin", bufs=1) as fin,
            tc.tile_pool(name="pst", bufs=2, space="PSUM") as pst,
            tc.tile_pool(name="psc", bufs=2, space="PSUM") as psc,
            tc.tile_pool(name="psg", bufs=2, space="PSUM") as psg,
            tc.tile_pool(name="psr", bufs=2, space="PSUM") as psr,
        ):
            # ---- constants ----
            ident = cst.tile([128, 128], f32)
            make_identity(nc, ident[:])

            cwt = cst.tile([128, K * NEC, FILT], f32r)
            nc.sync.dma_start(
                cwt[:EC, :, :], cwt_d[:].rearrange("kc e f -> e kc f").bitcast(f32r)
            )
            wsb = cst.tile([128, NFC, FILT], f32r)
            nc.sync.dma_start(
                wsb[:FC, :, :], w_d[:].rearrange("(c p) g -> p c g", p=FC).bitcast(f32r)
            )

            bsb = cst.tile([128, NFC], f32)
            nc.sync.dma_start(bsb[:FC, :], b_d[:].rearrange("(c p) -> p c", p=FC))

            zraw = cst.tile([128, 16], f32)
            nc.gpsimd.memset(zraw[:, :], 0.0)
            zeros_r = cst.tile([128, 16], f32r)
            nc.vector.tensor_copy(zeros_r[:, :], zraw[:, :])

            ones = cst.tile([128, 128], f32r)
            nc.sync.dma_start(ones[:, :], ones_d[:].bitcast(f32r))

            # pooled vectors in column layout: [f'(100p), sample, f-chunk]
            rq_col = fin.tile([128, BC, NFC], f32)
            ra_col = fin.tile([128, BC, NFC], f32)

            for grp in range(BC // GRP):
                # ---- gather + transpose question embeddings for 4 samples ----
                idx_q = io.tile([128, GRP], i32, tag="idxq")
                nc.sync.dma_start(
                    idx_q[:, :], q_d[GRP * grp : GRP * (grp + 1), :].rearrange("s p -> p s")
                )
                embq = io.tile([128, GRP, EMB], f32, tag="embq")
                for s in range(GRP):
                    nc.gpsimd.indirect_dma_start(
                        out=embq[:, s, :],
                        out_offset=None,
                        in_=emb_d[:],
                        in_offset=bass.IndirectOffsetOnAxis(ap=idx_q[:, s : s + 1], axis=0),
                    )
                # XTq[e', c, s, 1+l] = emb[q[s,l], 100c+e'] ; cols 0 and 129 zero pad
                xtq = xt.tile([128, NEC, GRP, LQ + 2], f32r, tag="xtq")
                zv = zeros_r[:EC, : NEC * GRP].rearrange("p (c s) -> p c s", c=NEC)
                nc.vector.tensor_copy(xtq[:EC, :, :, 0], zv)
                nc.vector.tensor_copy(xtq[:EC, :, :, LQ + 1], zv)
                for c in range(NEC):
                    ps_t = pst.tile([128, 512], f32, tag="pst")
                    for s in range(GRP):
                        nc.tensor.transpose(
                            ps_t[:EC, 128 * s : 128 * (s + 1)],
                            embq[:, s, EC * c : EC * (c + 1)],
                            ident[:],
                        )
                    nc.vector.tensor_copy(
                        xtq[:EC, c, :, 1 : LQ + 1],
                        ps_t[:EC, :].rearrange("p (s l) -> p s l", s=GRP),
                    )

                # ---- conv for 4 questions: Qg[f', ft, s, l] ----
                qg = enc.tile([128, NFC, GRP, LQ], f32r, tag="qg")
                for ft in range(NFC):
                    ps_c = psc.tile([128, 512], f32, tag="psc")
                    i = 0
                    for k in range(K):
                        for c in range(NEC):
                            nc.tensor.matmul(
                                ps_c[:FC, :].rearrange("p (s l) -> p s l", s=GRP),
                                lhsT=cwt[:EC, 3 * k + c, FC * ft : FC * ft + FC],
                                rhs=xtq[:EC, c, :, k : k + LQ],
                                start=(i == 0),
                                stop=(i == K * NEC - 1),
                            )
                            i += 1
                    nc.scalar.activation(
                        qg[:FC, ft, :, :],
                        ps_c[:FC, :].rearrange("p (s l) -> p s l", s=GRP),
                        AF.Identity,
                        bias=bsb[:FC, ft : ft + 1],
                        scale=1.0,
                    )

                # ---- Z1 = W^T Q for the group: z1[g', gt, s, l] ----
                z1 = enc.tile([128, NFC, GRP, LQ], f32r, tag="z1")
                for gt in range(NFC):
                    ps_z = psc.tile([128, 512], f32, tag="psc")
                    for c in range(NFC):
                        nc.tensor.matmul(
                            ps_z[:FC, :],
                            lhsT=wsb[:FC, c, FC * gt : FC * gt + FC],
                            rhs=qg[:FC, c, :, :],
                            start=(c == 0),
                            stop=(c == NFC - 1),
                        )
                    nc.vector.tensor_copy(
                        z1[:FC, gt, :, :],
                        ps_z[:FC, :].rearrange("p (s l) -> p s l", s=GRP),
                    )

                for s in range(GRP):
                    b = GRP * grp + s
                    # ---- gather + transpose answer embeddings ----
                    idx_a = io.tile([128, 4], i32, tag="idxa")
                    nc.sync.dma_start(
                        idx_a[:, :],
                        a_d[b : b + 1, :].rearrange("o (t p) -> p (o t)", p=128),
                    )
                    emba = io.tile([128, 4, EMB], f32, tag="emba")
                    for t in range(4):
                        nc.gpsimd.indirect_dma_start(
                            out=emba[:, t, :],
                            out_offset=None,
                            in_=emb_d[:],
                            in_offset=bass.IndirectOffsetOnAxis(ap=idx_a[:, t : t + 1], axis=0),
                        )
                    xta = xt.tile([128, NEC, LA + 2], f32r, tag="xta")
                    zva = zeros_r[:EC, :NEC].rearrange("p c -> p c")
                    nc.vector.tensor_copy(xta[:EC, :, 0], zva)
                    nc.vector.tensor_copy(xta[:EC, :, LA + 1], zva)
                    for c in range(NEC):
                        ps_t = pst.tile([128, 512], f32, tag="pst")
                        for t in range(4):
                            nc.tensor.transpose(
                                ps_t[:EC, 128 * t : 128 * (t + 1)],
                                emba[:, t, EC * c : EC * (c + 1)],
                                ident[:],
                            )
                        nc.vector.tensor_copy(xta[:EC, c, 1 : LA + 1], ps_t[:EC, :])

                    # ---- conv for answer: A[f', ft, a] ----
                    a_sb = enc.tile([128, NFC, LA], f32r, tag="asb")
                    for ft in range(NFC):
                        ps_c = psc.tile([128, 512], f32, tag="psc")
                        i = 0
                        for k in range(K):
                            for c in range(NEC):
                                nc.tensor.matmul(
                                    ps_c[:FC, :],
                                    lhsT=cwt[:EC, 3 * k + c, FC * ft : FC * ft + FC],
                                    rhs=xta[:EC, c, k : k + LA],
                                    start=(i == 0),
                                    stop=(i == K * NEC - 1),
                                )
                                i += 1
                        nc.scalar.activation(
                            a_sb[:FC, ft, :],
                            ps_c[:FC, :],
                            AF.Identity,
                            bias=bsb[:FC, ft : ft + 1],
                            scale=1.0,
                        )

                    # ---- G = tanh(Z1^T A) : [q=128, a=512] ----
                    ps_g = psg.tile([128, 512], f32, tag="psg")
                    for c in range(NFC):
                        nc.tensor.matmul(
                            ps_g[:, :],
                            lhsT=z1[:FC, c, s, :],
                            rhs=a_sb[:FC, c, :],
                            start=(c == 0),
                            stop=(c == NFC - 1),
                        )
                    gs = gp.tile([128, 512], f32, tag="gs")
                    nc.scalar.activation(gs[:, :], ps_g[:, :], AF.Tanh)

                    # ---- pooling weights ----
                    mq = gp.tile([128, 1], f32, tag="mq")
                    nc.vector.reduce_max(mq[:, :], gs[:, :], axis=AX)
                    wq2 = gp.tile([128, 2], f32r, tag="wq")
                    nc.scalar.activation(wq2[:, 0:1], mq[:, :], AF.Exp)
                    nc.scalar.activation(wq2[:, 1:2], mq[:, :], AF.Exp)

                    ps_gt = pst.tile([128, 512], f32, tag="pst")
                    for t in range(4):
                        nc.tensor.transpose(
                            ps_gt[:, 128 * t : 128 * (t + 1)],
                            gs[:, 128 * t : 128 * (t + 1)],
                            ident[:],
                        )
                    gt_sb = gp.tile([128, 4, 128], f32, tag="gt")
                    nc.vector.tensor_copy(
                        gt_sb[:, :, :], ps_gt[:, :].rearrange("p (t q) -> p t q", t=4)
                    )
                    ma = gp.tile([128, 4], f32, tag="ma")
                    nc.vector.reduce_max(ma[:, :], gt_sb[:, :, :], axis=AX)
                    wa2 = gp.tile([128, 4, 2], f32r, tag="wa")
                    nc.scalar.activation(wa2[:, :, 0], ma[:, :], AF.Exp)
                    nc.scalar.activation(wa2[:, :, 1], ma[:, :], AF.Exp)

                    # ---- A^T and Q^T for the pooled matvecs ----
                    at_sb = gp.tile([128, 4, FILT], f32r, tag="at")
                    for t in range(4):
                        ps_at = pst.tile([128, 512], f32, tag="pst")
                        for c in range(NFC):
                            nc.tensor.transpose(
                                ps_at[:, FC * c : FC * (c + 1)],
                                a_sb[:FC, c, 128 * t : 128 * (t + 1)].bitcast(f32),
                                ident[:FC, :FC],
                            )
                        nc.scalar.copy(at_sb[:, t, :], ps_at[:, :FILT])
                    qt_sb = gp.tile([128, FILT], f32r, tag="qt")
                    ps_qt = pst.tile([128, 512], f32, tag="pst")
                    for c in range(NFC):
                        nc.tensor.transpose(
                            ps_qt[:, FC * c : FC * (c + 1)],
                            qg[:FC, c, s, :].bitcast(f32),
                            ident[:FC, :FC],
                        )
                    nc.vector.tensor_copy(qt_sb[:, :], ps_qt[:, :FILT])

                    # ---- rA = A wA, rQ = Q wQ (unnormalized), column layout ----
                    ps_ra = psr.tile([128, 512], f32, tag="psr")
                    for t in range(4):
                        for c in range(NFC):
                            nc.tensor.matmul(
                                ps_ra[:FC, 2 * c : 2 * c + 2],
                                lhsT=at_sb[:, t, FC * c : FC * (c + 1)],
                                rhs=wa2[:, t, :],
                                start=(t == 0 and c == 0),
                                stop=(t == 3 and c == NFC - 1),
                            )
                    nc.vector.tensor_copy(
                        ra_col[:FC, b, :],
                        ps_ra[:FC, : 2 * NFC].rearrange("p (c two) -> p c two", two=2)[:, :, 0],
                    )
                    ps_rq = psr.tile([128, 512], f32, tag="psr")
                    for c in range(NFC):
                        nc.tensor.matmul(
                            ps_rq[:FC, 2 * c : 2 * c + 2],
                            lhsT=qt_sb[:, FC * c : FC * (c + 1)],
                            rhs=wq2[:, :],
                            start=(c == 0),
                            stop=(c == NFC - 1),
                        )
                    nc.vector.tensor_copy(
                        rq_col[:FC, b, :],
                        ps_rq[:FC, : 2 * NFC].rearrange("p (c two) -> p c two", two=2)[:, :, 0],
                    )
                    if DEBUG and b == DBG_B:
                        nc.sync.dma_start(xta_dbg[:, :, :], xta[:, :, :].bitcast(f32))
                        nc.sync.dma_start(asb_dbg[:, :, :], a_sb[:, :, :].bitcast(f32))
                        nc.sync.dma_start(gs_dbg[:, :], gs[:, :])
                        nc.sync.dma_start(wa_dbg[:, :], wa2[:, :, 0].bitcast(f32))
                        nc.sync.dma_start(wq_dbg[:, :], wq2[:, 0:1].bitcast(f32))
                        nc.sync.dma_start(at_dbg[:, :, :], at_sb[:, :, :].bitcast(f32))
                        nc.sync.dma_start(qt_dbg[:, :], qt_sb[:, :].bitcast(f32))
                        nc.sync.dma_start(qg_dbg[:, :, :, :], qg[:, :, :, :].bitcast(f32))
                        nc.sync.dma_start(z1_dbg[:, :, :, :], z1[:, :, :, :].bitcast(f32))


            if DEBUG:
                nc.sync.dma_start(rq_dbg[:, :, :], rq_col[:, :, :])
                nc.sync.dma_start(ra_dbg[:, :, :], ra_col[:, :, :])
            # ---- final cosine over the 16 samples ----
            prod = fin.tile([128, BC, NFC], f32r)
            nc.vector.tensor_tensor(
                out=prod[:FC, :, :], in0=rq_col[:FC, :, :], in1=ra_col[:FC, :, :],
                op=mybir.AluOpType.mult,
            )
            sqq = fin.tile([128, BC, NFC], f32r)
            nc.vector.tensor_tensor(
                out=sqq[:FC, :, :], in0=rq_col[:FC, :, :], in1=rq_col[:FC, :, :],
                op=mybir.AluOpType.mult,
            )
            sqa = fin.tile([128, BC, NFC], f32r)
            nc.vector.tensor_tensor(
                out=sqa[:FC, :, :], in0=ra_col[:FC, :, :], in1=ra_col[:FC, :, :],
                op=mybir.AluOpType.mult,
            )
            # contract f via all-ones matmul: out rows all equal the dot
            ps_f = psr.tile([128, 512], f32, tag="psr")
            for j, srcp in enumerate((prod, sqq, sqa)):
                for c in range(NFC):
                    nc.tensor.matmul(
                        ps_f[:, BC * j : BC * (j + 1)],
                        lhsT=ones[:FC, :],
                        rhs=srcp[:FC, :, c],
                        start=(j == 0 and c == 0),
                        stop=(j == 2 and c == NFC - 1),
                    )
            scal = fin.tile([1, 3 * BC], f32)
            nc.vector.tensor_copy(scal[:1, :], ps_f[:1, : 3 * BC])

            nrm = fin.tile([1, 2 * BC], f32)
            nc.scalar.activation(nrm[:1, :], scal[:1, BC : 3 * BC], AF.Sqrt)
            nc.vector.tensor_scalar_max(nrm[:1, :], nrm[:1, :], EPS)
            den = fin.tile([1, BC], f32)
            nc.vector.tensor_tensor(
                out=den[:1, :], in0=nrm[:1, :BC], in1=nrm[:1, BC:], op=mybir.AluOpType.mult
            )
            rec = fin.tile([1, BC], f32)
            nc.vector.reciprocal(rec[:1, :], den[:1, :])
            res = fin.tile([1, BC], f32)
            nc.vector.tensor_tensor(
                out=res[:1, :], in0=scal[:1, :BC], in1=rec[:1, :], op=mybir.AluOpType.mult
            )
            nc.sync.dma_start(out_d[:, :].rearrange("s o -> o s"), res[:1, :])

    nc.compile()
    return nc


_NC_CACHE = {}


def _get_nc():
    if "nc" not in _NC_CACHE:
        _NC_CACHE["nc"] = _build()
    return _NC_CACHE["nc"]


_ONES = np.ones((128, 128), dtype=np.float32)


def _make_in_maps(questions, answers, emb, conv_w, conv_b, W):
    # cwt[3k+c, e', f] = conv_w[f, 100c+e', k]
    cwt = np.ascontiguousarray(
        conv_w.transpose(2, 1, 0).reshape(K, NEC, EC, FILT).reshape(K * NEC, EC, FILT)
    ).astype(np.float32)
    emb = np.ascontiguousarray(emb, dtype=np.float32)
    W = np.ascontiguousarray(W, dtype=np.float32)
    conv_b = np.ascontiguousarray(conv_b, dtype=np.float32)
    in_maps = []
    for c in range(NCORES):
        in_maps.append(
            {
                "questions": np.ascontiguousarray(questions[BC * c : BC * (c + 1)], dtype=np.int32),
                "answers": np.ascontiguousarray(answers[BC * c : BC * (c + 1)], dtype=np.int32),
                "emb": emb,
                "cwt": cwt,
                "W": W,
                "conv_b": conv_b,
                "ones": _ONES,
            }
        )
    return in_maps


def run(questions, answers, emb, conv_w, conv_b, W, trace=False):
    nc = _get_nc()
    in_maps = _make_in_maps(questions, answers, emb, conv_w, conv_b, W)
    res = run_bass_kernel_spmd(nc, in_maps, list(range(NCORES)), trace=trace)
    out = np.concatenate([res.results[c]["out"][:, 0] for c in range(NCORES)])
    return out.astype(np.float32), res


def kernel(questions, answers, emb, conv_w, conv_b, W):
    out, _ = run(questions, answers, emb, conv_w, conv_b, W)
    return out


# revision 13
# speedup vs baseline: 1.0922x; 1.0922x over previous
"""AttentivePoolingNetwork TRN2 kernel.

Data-parallel over batch across 8 NeuronCores (16 samples/core).
Per sample:
  Q = conv1d(emb[questions]) + b    [F=400, LQ=128]   (fp32r matmuls)
  A = conv1d(emb[answers]) + b      [F=400, LA=512]
  G = tanh(Q^T W A)                 [128, 512]
  wQ = exp(max_a G)  (softmax normalization cancels in the final cosine)
  wA = exp(max_q G)
  out = cos(Q wQ, A wA)
"""
import numpy as np
import concourse.bass as bass
import concourse.tile as tile
from concourse import bacc, mybir
from concourse.bass_utils import run_bass_kernel_spmd

f32 = mybir.dt.float32
f32r = mybir.dt.float32r
i32 = mybir.dt.int32
AX = mybir.AxisListType.X
AF = mybir.ActivationFunctionType

B, LQ, LA = 128, 128, 512
VOCAB, EMB, FILT, K = 50000, 300, 400, 3
NCORES = 8
BC = B // NCORES        # 16 samples per core
EC = 100                # e-chunk size -> 3 chunks
FC = 100                # f-chunk size -> 4 chunks
NEC = EMB // EC         # 3
NFC = FILT // FC        # 4
GRP = 4                 # samples per Q-group (packs Q-conv/Z1 to N=512)
EPS = 1e-8
DEBUG = False
DBG_B = 1


def _build():
    nc = bacc.Bacc("TRN2", target_bir_lowering=False, debug=False, num_devices=NCORES)

    q_d = nc.declare_dram_parameter("questions", [BC, LQ], i32, isOutput=False)
    a_d = nc.declare_dram_parameter("answers", [BC, LA], i32, isOutput=False)
    emb_d = nc.declare_dram_parameter("emb", [VOCAB + 1, EMB], f32, isOutput=False)
    # host-prepped: cwt[k*3+c, e', f] = conv_w[f, 100c+e', k]
    cwt_d = nc.declare_dram_parameter("cwt", [K * NEC, EC, FILT], f32, isOutput=False)
    w_d = nc.declare_dram_parameter("W", [FILT, FILT], f32, isOutput=False)
    b_d = nc.declare_dram_parameter("conv_b", [FILT], f32, isOutput=False)
    ones_d = nc.declare_dram_parameter("ones", [128, 128], f32, isOutput=False)
    ident_d = nc.declare_dram_parameter("ident", [128, 128], f32, isOutput=False)
    out_d = nc.declare_dram_parameter("out", [BC, 1], f32, isOutput=True)
    if DEBUG:
        xta_dbg = nc.declare_dram_parameter("xta_dbg", [128, NEC, LA + 2], f32, isOutput=True)
        asb_dbg = nc.declare_dram_parameter("asb_dbg", [128, NFC, LA], f32, isOutput=True)
        gs_dbg = nc.declare_dram_parameter("gs_dbg", [128, 512], f32, isOutput=True)
        wa_dbg = nc.declare_dram_parameter("wa_dbg", [128, 4], f32, isOutput=True)
        wq_dbg = nc.declare_dram_parameter("wq_dbg", [128, 1], f32, isOutput=True)
        rq_dbg = nc.declare_dram_parameter("rq_dbg", [128, BC, NFC], f32, isOutput=True)
        ra_dbg = nc.declare_dram_parameter("ra_dbg", [128, BC, NFC], f32, isOutput=True)
        qg_dbg = nc.declare_dram_parameter("qg_dbg", [128, NFC, GRP, LQ], f32, isOutput=True)
        z1_dbg = nc.declare_dram_parameter("z1_dbg", [128, NFC, GRP, LQ], f32, isOutput=True)

    with tile.TileContext(nc) as tc:
        with (
            tc.tile_pool(name="const", bufs=1) as cst,
            tc.tile_pool(name="io", bufs=2) as io,
            tc.tile_pool(name="xt", bufs=2) as xt,
            tc.tile_pool(name="enc", bufs=2) as enc,
            tc.tile_pool(name="gp", bufs=2) as gp,
            tc.tile_pool(name="fin", bufs=1) as fin,
            tc.tile_pool(name="pst", bufs=2, space="PSUM") as pst,
            tc.tile_pool(name="psc", bufs=2, space="PSUM") as psc,
            tc.tile_pool(name="psg", bufs=2, space="PSUM") as psg,
            tc.tile_pool(name="psr", bufs=2, space="PSUM") as psr,
        ):
            # ---- constants ----
            ident = cst.tile([128, 128], f32r)
            nc.sync.dma_start(ident[:, :], ident_d[:].bitcast(f32r))

            cwt = cst.tile([128, K * NEC, FILT], f32r)
            nc.sync.dma_start(
                cwt[:EC, :, :], cwt_d[:].rearrange("kc e f -> e kc f").bitcast(f32r)
            )
            wsb = cst.tile([128, NFC, FILT], f32r)
            nc.sync.dma_start(
                wsb[:FC, :, :], w_d[:].rearrange("(c p) g -> p c g", p=FC).bitcast(f32r)
            )

            bsb = cst.tile([128, NFC], f32)
            nc.sync.dma_start(bsb[:FC, :], b_d[:].rearrange("(c p) -> p c", p=FC))

            zraw = cst.tile([128, 16], f32)
            nc.gpsimd.memset(zraw[:, :], 0.0)
            zeros_r = cst.tile([128, 16], f32r)
            nc.vector.tensor_copy(zeros_r[:, :], zraw[:, :])

            ones = cst.tile([128, 128], f32r)
            nc.sync.dma_start(ones[:, :], ones_d[:].bitcast(f32r))

            # pooled vectors in column layout: [f'(100p), sample, f-chunk]
            rq_col = fin.tile([128, BC, NFC], f32)
            ra_col = fin.tile([128, BC, NFC], f32)

            for grp in range(BC // GRP):
                # ---- gather + transpose question embeddings for 4 samples ----
                idx_q = io.tile([128, GRP], i32, tag="idxq")
                nc.sync.dma_start(
                    idx_q[:, :], q_d[GRP * grp : GRP * (grp + 1), :].rearrange("s p -> p s")
                )
                embq = io.tile([128, GRP, EMB], f32r, tag="embq")
                for s in range(GRP):
                    nc.gpsimd.indirect_dma_start(
                        out=embq[:, s, :],
                        out_offset=None,
                        in_=emb_d[:].bitcast(f32r),
                        in_offset=bass.IndirectOffsetOnAxis(ap=idx_q[:, s : s + 1], axis=0),
                    )
                # XTq[e', c, s, 1+l] = emb[q[s,l], 100c+e'] ; cols 0 and 129 zero pad
                xtq = xt.tile([128, NEC, GRP, LQ + 2], f32r, tag="xtq")
                zv = zeros_r[:EC, : NEC * GRP].rearrange("p (c s) -> p c s", c=NEC)
                nc.vector.tensor_copy(xtq[:EC, :, :, 0], zv)
                nc.vector.tensor_copy(xtq[:EC, :, :, LQ + 1], zv)
                for c in range(NEC):
                    ps_t = pst.tile([128, 512], f32r, tag="pst")
                    for s in range(GRP):
                        nc.tensor.transpose(
                            ps_t[:EC, 128 * s : 128 * (s + 1)],
                            embq[:, s, EC * c : EC * (c + 1)],
                            ident[:],
                        )
                    nc.vector.tensor_copy(
                        xtq[:EC, c, :, 1 : LQ + 1],
                        ps_t[:EC, :].rearrange("p (s l) -> p s l", s=GRP),
                    )

                # ---- conv for 4 questions: Qg[f', ft, s, l] ----
                qg = enc.tile([128, NFC, GRP, LQ], f32r, tag="qg")
                for ft in range(NFC):
                    ps_c = psc.tile([128, 512], f32, tag="psc")
                    i = 0
                    for k in range(K):
                        for c in range(NEC):
                            nc.tensor.matmul(
                                ps_c[:FC, :].rearrange("p (s l) -> p s l", s=GRP),
                                lhsT=cwt[:EC, 3 * k + c, FC * ft : FC * ft + FC],
                                rhs=xtq[:EC, c, :, k : k + LQ],
                                start=(i == 0),
                                stop=(i == K * NEC - 1),
                            )
                            i += 1
                    nc.scalar.activation(
                        qg[:FC, ft, :, :],
                        ps_c[:FC, :].rearrange("p (s l) -> p s l", s=GRP),
                        AF.Identity,
                        bias=bsb[:FC, ft : ft + 1],
                        scale=1.0,
                    )

                # ---- Z1 = W^T Q for the group: z1[g', gt, s, l] ----
                z1 = enc.tile([128, NFC, GRP, LQ], f32r, tag="z1")
                for gt in range(NFC):
                    ps_z = psc.tile([128, 512], f32, tag="psc")
                    for c in range(NFC):
                        nc.tensor.matmul(
                            ps_z[:FC, :],
                            lhsT=wsb[:FC, c, FC * gt : FC * gt + FC],
                            rhs=qg[:FC, c, :, :],
                            start=(c == 0),
                            stop=(c == NFC - 1),
                        )
                    nc.vector.tensor_copy(
                        z1[:FC, gt, :, :],
                        ps_z[:FC, :].rearrange("p (s l) -> p s l", s=GRP),
                    )

                for s in range(GRP):
                    b = GRP * grp + s
                    # ---- gather + transpose answer embeddings ----
                    idx_a = io.tile([128, 4], i32, tag="idxa")
                    nc.sync.dma_start(
                        idx_a[:, :],
                        a_d[b : b + 1, :].rearrange("o (t p) -> p (o t)", p=128),
                    )
                    emba = io.tile([128, 4, EMB], f32r, tag="emba")
                    for t in range(4):
                        nc.gpsimd.indirect_dma_start(
                            out=emba[:, t, :],
                            out_offset=None,
                            in_=emb_d[:].bitcast(f32r),
                            in_offset=bass.IndirectOffsetOnAxis(ap=idx_a[:, t : t + 1], axis=0),
                        )
                    xta = xt.tile([128, NEC, LA + 2], f32r, tag="xta")
                    zva = zeros_r[:EC, :NEC].rearrange("p c -> p c")
                    nc.vector.tensor_copy(xta[:EC, :, 0], zva)
                    nc.vector.tensor_copy(xta[:EC, :, LA + 1], zva)
                    for c in range(NEC):
                        ps_t = pst.tile([128, 512], f32r, tag="pst")
                        for t in range(4):
                            nc.tensor.transpose(
                                ps_t[:EC, 128 * t : 128 * (t + 1)],
                                emba[:, t, EC * c : EC * (c + 1)],
                                ident[:],
                            )
                        nc.vector.tensor_copy(xta[:EC, c, 1 : LA + 1], ps_t[:EC, :])

                    # ---- conv for answer: A[f', ft, a] ----
                    a_sb = enc.tile([128, NFC, LA], f32r, tag="asb")
                    for ft in range(NFC):
                        ps_c = psc.tile([128, 512], f32, tag="psc")
                        i = 0
                        for k in range(K):
                            for c in range(NEC):
                                nc.tensor.matmul(
                                    ps_c[:FC, :],
                                    lhsT=cwt[:EC, 3 * k + c, FC * ft : FC * ft + FC],
                                    rhs=xta[:EC, c, k : k + LA],
                                    start=(i == 0),
                                    stop=(i == K * NEC - 1),
                                )
                                i += 1
                        nc.scalar.activation(
                            a_sb[:FC, ft, :],
                            ps_c[:FC, :],
                            AF.Identity,
                            bias=bsb[:FC, ft : ft + 1],
                            scale=1.0,
                        )

                    # ---- G = tanh(Z1^T A) : [q=128, a=512] ----
                    ps_g = psg.tile([128, 512], f32, tag="psg")
                    for c in range(NFC):
                        nc.tensor.matmul(
                            ps_g[:, :],
                            lhsT=z1[:FC, c, s, :],
                            rhs=a_sb[:FC, c, :],
                            start=(c == 0),
                            stop=(c == NFC - 1),
                        )
                    gs = gp.tile([128, 512], f32r, tag="gs")
                    nc.scalar.activation(gs[:, :], ps_g[:, :], AF.Tanh)

                    # ---- pooling weights ----
                    mq = gp.tile([128, 1], f32, tag="mq")
                    nc.vector.reduce_max(mq[:, :], gs[:, :], axis=AX)
                    wqp = gp.tile([128, 1], f32, tag="wq")
                    nc.scalar.activation(wqp[:, :], mq[:, :], AF.Exp)
                    wqf = gp.tile([1, 128], f32r, tag="wqf")
                    nc.sync.dma_start(wqf[:1, :], wqp[:, :].bitcast(f32r))

                    ps_gt = pst.tile([128, 512], f32r, tag="pst")
                    for t in range(4):
                        nc.tensor.transpose(
                            ps_gt[:, 128 * t : 128 * (t + 1)],
                            gs[:, :].rearrange("p (l four) -> p four l", four=4)[:, t, :],
                            ident[:],
                        )
                    gt_sb = gp.tile([128, 4, 128], f32, tag="gt")
                    nc.vector.tensor_copy(
                        gt_sb[:, :, :], ps_gt[:, :].rearrange("p (t q) -> p t q", t=4)
                    )
                    ma = gp.tile([128, 4], f32, tag="ma")
                    nc.vector.reduce_max(ma[:, :], gt_sb[:, :, :], axis=AX)
                    wap = gp.tile([128, 4], f32, tag="wa")
                    nc.scalar.activation(wap[:, :], ma[:, :], AF.Exp)
                    waf = gp.tile([1, 512], f32r, tag="waf")
                    nc.sync.dma_start(
                        waf[:1, :].rearrange("o (p t) -> o p t", p=128),
                        wap[:, :].bitcast(f32r),
                    )

                    # broadcast weights across partitions via K=1 ones-matmul
                    ps_ba = psr.tile([128, 512], f32, tag="psr")
                    nc.tensor.matmul(
                        ps_ba[:, :], lhsT=ones[:1, :], rhs=waf[:1, :],
                        start=True, stop=True,
                    )
                    ps_bq = psr.tile([128, 512], f32, tag="psr")
                    nc.tensor.matmul(
                        ps_bq[:, :LQ], lhsT=ones[:1, :], rhs=wqf[:1, :],
                        start=True, stop=True,
                    )

                    # rA[f] = sum_a A[f,a] wA[a]; rQ[f] = sum_q Q[f,q] wQ[q]
                    for c in range(NFC):
                        tmp = gp.tile([128, 512], f32, tag="rtmp")
                        nc.vector.tensor_tensor(
                            out=tmp[:FC, :], in0=a_sb[:FC, c, :].bitcast(f32),
                            in1=ps_ba[:FC, :], op=mybir.AluOpType.mult,
                        )
                        nc.vector.reduce_sum(ra_col[:FC, b, c : c + 1], tmp[:FC, :], axis=AX)
                        tmpq = gp.tile([128, 128], f32, tag="rtmpq")
                        nc.vector.tensor_tensor(
                            out=tmpq[:FC, :], in0=qg[:FC, c, s, :].bitcast(f32),
                            in1=ps_bq[:FC, :LQ], op=mybir.AluOpType.mult,
                        )
                        nc.vector.reduce_sum(rq_col[:FC, b, c : c + 1], tmpq[:FC, :], axis=AX)

                    if DEBUG and b == DBG_B:
                        nc.sync.dma_start(xta_dbg[:, :, :], xta[:, :, :].bitcast(f32))
                        nc.sync.dma_start(asb_dbg[:, :, :], a_sb[:, :, :].bitcast(f32))
                        nc.sync.dma_start(gs_dbg[:, :], gs[:, :].bitcast(f32))
                        nc.sync.dma_start(wa_dbg[:, :], wap[:, :])
                        nc.sync.dma_start(wq_dbg[:, :], wqp[:, :])
                        nc.sync.dma_start(qg_dbg[:, :, :, :], qg[:, :, :, :].bitcast(f32))
                        nc.sync.dma_start(z1_dbg[:, :, :, :], z1[:, :, :, :].bitcast(f32))


            if DEBUG:
                nc.sync.dma_start(rq_dbg[:, :, :], rq_col[:, :, :])
                nc.sync.dma_start(ra_dbg[:, :, :], ra_col[:, :, :])
            # ---- final cosine over the 16 samples ----
            prod = fin.tile([128, BC, NFC], f32r)
            nc.vector.tensor_tensor(
                out=prod[:FC, :, :], in0=rq_col[:FC, :, :], in1=ra_col[:FC, :, :],
                op=mybir.AluOpType.mult,
            )
            sqq = fin.tile([128, BC, NFC], f32r)
            nc.vector.tensor_tensor(
                out=sqq[:FC, :, :], in0=rq_col[:FC, :, :], in1=rq_col[:FC, :, :],
                op=mybir.AluOpType.mult,
            )
            sqa = fin.tile([128, BC, NFC], f32r)
            nc.vector.tensor_tensor(
                out=sqa[:FC, :, :], in0=ra_col[:FC, :, :], in1=ra_col[:FC, :, :],
                op=mybir.AluOpType.mult,
            )
            # contract f via all-ones matmul: out rows all equal the dot
            ps_f = psr.tile([128, 512], f32, tag="psr")
            for j, srcp in enumerate((prod, sqq, sqa)):
                for c in range(NFC):
                    nc.tensor.matmul(
                        ps_f[:, BC * j : BC * (j + 1)],
                        lhsT=ones[:FC, :],
                        rhs=srcp[:FC, :, c],
                        start=(j == 0 and c == 0),
                        stop=(j == 2 and c == NFC - 1),
                    )
            scal = fin.tile([1, 3 * BC], f32)
            nc.vector.tensor_copy(scal[:1, :], ps_f[:1, : 3 * BC])

            nrm = fin.tile([1, 2 * BC], f32)
            nc.scalar.activation(nrm[:1, :], scal[:1, BC : 3 * BC], AF.Sqrt)
            nc.vector.tensor_scalar_max(nrm[:1, :], nrm[:1, :], EPS)
            den = fin.tile([1, BC], f32)
            nc.vector.tensor_tensor(
                out=den[:1, :], in0=nrm[:1, :BC], in1=nrm[:1, BC:], op=mybir.AluOpType.mult
            )
            rec = fin.tile([1, BC], f32)
            nc.vector.reciprocal(rec[:1, :], den[:1, :])
            res = fin.tile([1, BC], f32)
            nc.vector.tensor_tensor(
                out=res[:1, :], in0=scal[:1, :BC], in1=rec[:1, :], op=mybir.AluOpType.mult
            )
            nc.sync.dma_start(out_d[:, :].rearrange("s o -> o s"), res[:1, :])

    nc.compile()
    return nc


_NC_CACHE = {}


def _get_nc():
    if "nc" not in _NC_CACHE:
        _NC_CACHE["nc"] = _build()
    return _NC_CACHE["nc"]


_ONES = np.ones((128, 128), dtype=np.float32)
_IDENT = np.eye(128, dtype=np.float32)


def _make_in_maps(questions, answers, emb, conv_w, conv_b, W):
    # cwt[3k+c, e', f] = conv_w[f, 100c+e', k]
    cwt = np.ascontiguousarray(
        conv_w.transpose(2, 1, 0).reshape(K, NEC, EC, FILT).reshape(K * NEC, EC, FILT)
    ).astype(np.float32)
    emb = np.ascontiguousarray(emb, dtype=np.float32)
    W = np.ascontiguousarray(W, dtype=np.float32)
    conv_b = np.ascontiguousarray(conv_b, dtype=np.float32)
    in_maps = []
    for c in range(NCORES):
        in_maps.append(
            {
                "questions": np.ascontiguousarray(questions[BC * c : BC * (c + 1)], dtype=np.int32),
                "answers": np.ascontiguousarray(answers[BC * c : BC * (c + 1)], dtype=np.int32),
                "emb": emb,
                "cwt": cwt,
                "W": W,
                "conv_b": conv_b,
                "ones": _ONES,
                "ident": _IDENT,
            }
        )
    return in_maps


def run(questions, answers, emb, conv_w, conv_b, W, trace=False):
    nc = _get_nc()
    in_maps = _make_in_maps(questions, answers, emb, conv_w, conv_b, W)
    res = run_bass_kernel_spmd(nc, in_maps, list(range(NCORES)), trace=trace)
    out = np.concatenate([res.results[c]["out"][:, 0] for c in range(NCORES)])
    return out.astype(np.float32), res


def kernel(questions, answers, emb, conv_w, conv_b, W):
    out, _ = run(questions, answers, emb, conv_w, conv_b, W)
    return out
